# revision 57
# baseline (speedup 1.0000x reference)
"""CodecAttention (sliding-window attention w/ QK-RMSNorm + ALiBi) on 8 trn2 cores.

Sharding: data-parallel over (batch, sequence-chunk): 2 batches x 4 chunks of 512
queries -> 8 cores. Each core recomputes K/V for its 512-token halo (zero-padded
for the first chunk), so there is no cross-core communication; the host only
slices/transposes inputs and concatenates the 8 disjoint output slices.

On-core pipeline (bf16 operands, fp32 PSUM accumulation):
  A) QKV projections from x^T (dim-on-partitions), dc-outer so compute
     starts ~256KB into the DMA stream; startup DMAs ride both hwdge rings
     (scalar ring reserved for wq0 so its eviction ACTs are never queued
     behind descriptor generation). QK RMSNorm over the flat 1024-dim axis
     via ACT-square + accumulating ones-matmul partition reduction (sq-acc
     on gpsimd), rsqrt as exp(-0.5*ln(x)), 1/sqrt(dh) folded into the q
     scale; the normalize stts interleave through the K/V matmul windows.
  B) Head pairs, software-pipelined (scores(hp) issue before AV(hp-1) so
     the ACT exp chain - the phase bottleneck - never starves): S^T tiles =
     k^T.T @ q^T as a 68-row contraction per head (64 head dims + 4 ALiBi
     rows: hi/lo bf16-split position terms; padded halo keys of chunk 0
     killed via a -3e4 key bias), packed tightly as the 20 valid 128-col
     blocks per head (bank-aligned, no dead rectangles), ACT exp with a
     fixed offset straight off PSUM, gpsimd affine_select zeroes the
     causal edge triangles (window edges only for heads 12-15; below that
     ALiBi decay exp(-slope*W) <= 3e-4 makes them negligible), AV+rowsum
     with V-as-stationary (ones column yields softmax denominators),
     denominators DMA'd from PSUM into an [8, TQ] tile, one grouped
     reciprocal, K=8 selection-matrix matmul broadcasts the reciprocals
     per pair, normalize muls spread one pair per hp.
  C) out = attnT.T @ wo^T per token tile; three tile-fronts bridge group
     1's reciprocal chain, finished halves stream out on both hwdge rings.
"""

import contextlib
import ctypes
import os
import sys
import types

import ml_dtypes
import numpy as np

import concourse.bass as bass
import concourse.mybir as mybir
import concourse.tile as tile


def _install_axon_ntff_shim():
    """bass_utils' trace path wants antenv.axon_hooks, which this image lacks.
    Provide it, backed by direct ctypes calls into libaxon_pjrt.so (same ABI
    the agent boot would use). Degrades to hook=None if the .so is absent."""
    try:
        import antenv.axon_hooks  # noqa: F401
        return
    except ImportError:
        pass

    _hook_holder = [None]
    so_path = "/opt/axon/libaxon_pjrt.so"
    if os.path.exists(so_path):
        try:
            lib = ctypes.CDLL(so_path)
            if hasattr(lib, "axon_start_nrt_profile"):
                lib.axon_start_nrt_profile.argtypes = [
                    ctypes.POINTER(ctypes.c_int64), ctypes.c_size_t]
                lib.axon_start_nrt_profile.restype = ctypes.c_int64
                lib.axon_stop_nrt_profile.argtypes = [ctypes.c_char_p]
                lib.axon_stop_nrt_profile.restype = ctypes.c_int64

                @contextlib.contextmanager
                def _hook(output_dir, device_ids):
                    import jax
                    jax.devices()
                    if device_ids:
                        ids = (ctypes.c_int64 * len(device_ids))(*device_ids)
                        rc = lib.axon_start_nrt_profile(ids, len(device_ids))
                    else:
                        rc = lib.axon_start_nrt_profile(None, 0)
                    if rc != 0:
                        raise RuntimeError(f"axon_start_nrt_profile rc={rc}")
                    try:
                        yield
                    finally:
                        n = lib.axon_stop_nrt_profile(str(output_dir).encode())
                        if n < 0:
                            raise RuntimeError(f"axon_stop_nrt_profile rc={n}")

                _hook_holder[0] = _hook
        except OSError:
            pass

    mod = types.ModuleType("antenv.axon_hooks")
    mod.get_axon_ntff_profile_hook = lambda: _hook_holder[0]
    mod.set_axon_ntff_profile_hook = lambda h: _hook_holder.__setitem__(0, h)
    sys.modules["antenv.axon_hooks"] = mod


_install_axon_ntff_shim()

from concourse.bass_utils import run_bass_kernel_spmd  # noqa: E402
from bass_rust import ScopedClock  # noqa: E402

B, T, DIM = 2, 2048, 1024
H, DH, WINDOW = 16, 64, 512
P = 128
TQ = 512            # queries per core
TKV = 1024          # kv tokens per core (incl. 512 halo)
NCORES = 8
NQT = TQ // P       # 4
NKT = TKV // P      # 8
NDC = DIM // P      # 8
EXP_C = 10.0        # exp offset; true max masked score is ~6.0 for this data
F32 = mybir.dt.float32
F32R = mybir.dt.float32r
BF16 = mybir.dt.bfloat16
PREC = os.environ.get("KERNEL_PREC", "bf16")
DT = F32R if PREC == "fp32r" else BF16
AF = mybir.ActivationFunctionType
ALU = mybir.AluOpType

SLOPES = [2.0 ** (-0.5 * (h + 1)) for h in range(H)]

# Score-tile packing: per head, scores are computed as S^T [key, query] in two
# 3-bank PSUM halves of [128, 1536] with only cols [0, 1280) used. Key-tile kt
# covers queries [KT_QLO[kt], KT_QLO[kt]+KT_W[kt]) at column offset KT_OFF[kt]
# of its half. Tight packing: exactly the 20 valid (kt, qb) blocks
# (qb <= kt <= qb+4), every piece within one 2KB PSUM bank:
#   half0 cols: [kt3 512][kt2 384 | kt0 128][kt1 256 | 256 unused]
#   half1 cols: [kt4 512][kt5 384 | kt7 128][kt6 256 | 256 unused]
KT_W = [128, 256, 384, 512, 512, 384, 256, 128]
KT_OFF = [896, 1024, 512, 0, 0, 512, 1024, 896]
KT_QLO = [0, 0, 0, 0, 0, 128, 256, 384]
HW_HALF = 1536
HW_USED = 1280
# post-exp boundary cleanup per half: (col0, kind); kt==qb is the window edge
# ('win': keep key>=query within the block), kt==qb+4 the causal edge
# ('causal': keep query>=key). Tight packing leaves no dead blocks.
PT_FIX = {
    0: [(384, "win"), (768, "win"), (896, "win"), (1152, "win")],
    1: [(0, "causal"), (512, "causal"), (896, "causal"), (1024, "causal")],
}
# merged AV matmul plan: (kt, q_lo, width) at pt col KT_OFF+q_lo-KT_QLO; the
# first, full-width matmul initializes every PSUM element (per-element
# has_written handles the partial-region accumulation). Half-0 key tiles
# lead so AV streams while half 1's exp/fixups are still finishing.
AV_PLAN = [
    (3, 0, 512), (2, 0, 384), (1, 0, 256), (0, 0, 128),
    (4, 0, 512), (5, 128, 384), (6, 256, 256), (7, 384, 128),
]


class _SplitDrainTileContext(tile.TileContext):
    """The walrus build in this env rejects >1-2 sync-wait commands on one
    instruction; spread excess waits across same-engine NOPs placed directly
    before the over-limit instruction (per-engine program order preserved)."""

    def _split_excess_waits(self):
        nc = self.nc
        cur_list = nc.cur_bb.bb.instructions
        for blk in nc.m.functions[0].blocks:
            snapshot = list(blk.instructions)
            for inst in snapshot:
                si = inst.sync_info
                max_w = 1
                if si is None or len(si.on_wait) <= max_w:
                    continue
                waits = list(si.on_wait)
                si.on_wait = waits[:max_w]
                eng_obj = nc.engines[inst.engine]
                for w in waits[max_w:]:
                    nop_bi = eng_obj.nop(nofuse=True, hint="wait_split")
                    nop_inst = nop_bi.ins
                    nop_inst.sync_info = mybir.SyncInfo(on_wait=[w], on_update=[])
                    cur_list.remove(nop_inst)
                    blk.instructions.insert(
                        blk.instructions.index(inst), nop_inst)

    def _drain_and_barrier(self, tick_clock, wait_clock):
        self._split_excess_waits()
        drain_inst = self.nc.sync.drain()
        wait_clock.add_sem_waits(
            drain_inst.ins, ScopedClock({None: tick_clock.global_clock})
        )
        si = drain_inst.ins.sync_info
        if si is not None and len(si.on_wait) > 1:
            waits = list(si.on_wait)
            si.on_wait = waits[:1]
            for w in waits[1:]:
                nop = self.nc.sync.nop(nofuse=True, hint="drain_wait_split")
                nop.ins.sync_info = mybir.SyncInfo(on_wait=[w], on_update=[])
        self.nc.all_engine_barrier()
        assert self.sems is not None
        popped = self.nc._tile_sem_poison_stack.pop()
        assert popped is self._sem_poison
        self.nc.clear_and_free_semaphores(list(self.sems.allocated().values()))
        self.nc.all_engine_barrier()


def _src_nonce():
    import zlib
    with open(__file__, "rb") as f:
        return (zlib.crc32(f.read() + PREC.encode()) % 2048) + 8


def _pt_fixups(nc, pt, half, hp):
    """Zero the invalid regions of a post-exp pt tile on the gpsimd engine.

    Window-edge blocks are skipped for heads 0-11 (hp <= 5): their ALiBi
    slope satisfies slope*WINDOW >= 8, so the just-outside-window entries
    carry exp(-slope*WINDOW) <= 3e-4 of a typical in-window weight and are
    numerically negligible against the ~300-term softmax denominator."""
    for col0, kind in PT_FIX[half]:
        sl = pt[:, col0:col0 + P]
        if kind == "dead":
            nc.gpsimd.memset(sl, 0.0)
        elif kind == "win":
            if hp <= 5:
                continue
            # keep iff key_local >= query_local  (iota = p - qi >= 0)
            nc.gpsimd.affine_select(
                out=sl, in_=sl, compare_op=ALU.is_ge, fill=0.0,
                base=0, pattern=[[-1, P]], channel_multiplier=1)
        else:
            # causal: keep iff query_local >= key_local (iota = qi - p >= 0)
            nc.gpsimd.affine_select(
                out=sl, in_=sl, compare_op=ALU.is_ge, fill=0.0,
                base=0, pattern=[[1, P]], channel_multiplier=-1)


def _build_program(debug=False):
    nc = bass.Bass()
    # dummy input whose shape changes with this file: busts HLO-keyed NEFF
    # caches (the BIR itself is not part of the HLO fingerprint)
    nonce = nc.declare_dram_parameter("nonce", [1, _src_nonce()], F32,
                                      isOutput=False)
    xT = nc.declare_dram_parameter("xT", [DIM, TKV], DT, isOutput=False)
    # weights pre-permuted on host to [partition, slice, dc, 512] so every
    # DMA line is one contiguous 8KB-per-partition segment
    wT = nc.declare_dram_parameter("wT", [P, 6, NDC, 512], DT, isOutput=False)
    woT = nc.declare_dram_parameter("woT", [P, 2, NDC, 512], DT, isOutput=False)
    qext = nc.declare_dram_parameter("qext", [4, H, TQ], DT, isOutput=False)
    kext = nc.declare_dram_parameter("kext", [4, H, TKV], DT, isOutput=False)
    gam = nc.declare_dram_parameter("gam", [DH, 2 * H], F32, isOutput=False)
    out = nc.declare_dram_parameter("out", [TQ, DIM], F32, isOutput=True)
    if debug:
        qT_d = nc.declare_dram_parameter("qT_d", [P, NDC, TQ], DT, isOutput=True)
        kT_d = nc.declare_dram_parameter("kT_d", [P, NDC, TKV], DT, isOutput=True)
        V_d = nc.declare_dram_parameter("V_d", [P, NKT, H, DH + 1], DT, isOutput=True)
        aT_d = nc.declare_dram_parameter("aT_d", [P, NDC, TQ], DT, isOutput=True)
        pt_d = nc.declare_dram_parameter("pt_d", [2, P, 2, HW_HALF], DT, isOutput=True)

    with _SplitDrainTileContext(nc) as tc, \
            tc.tile_pool(name="persist", bufs=1) as pp, \
            tc.tile_pool(name="small", bufs=1) as psm:

        # per-head score operands: rows 0..63 = head dims, rows 64..67 = the
        # ALiBi rank-4 rows (folded into the same 68-partition contraction)
        qT = pp.tile([P, H, TQ], DT, tag="qT")         # [p, h, tok]
        kT = pp.tile([P, H, TKV], DT, tag="kT")
        V = pp.tile([P, NKT, H, DH + 1], DT, tag="V")  # [p=tok, kt, h, dh+ones]
        attnT = pp.tile([P, NDC, TQ], DT, tag="attnT")
        gam_sb = pp.tile([DH, 2 * H], F32, tag="gam")
        ones_sb = pp.tile([P, 1], F32R, tag="ones")
        ones_row = pp.tile([1, P], F32, tag="onesrow")
        negc_sb = pp.tile([P, 1], F32, tag="negc")
        eps_sb = pp.tile([1, 1], F32, tag="eps")
        ln8_sb = pp.tile([1, 1], F32, tag="ln8")
        # per-pair selection matrices for the reciprocal broadcast matmul:
        # sel[j][g, p] = 1 iff g == 2j + p//64   (K=8 stationary, bf16)
        sel_sb = pp.tile([8, 4, P], BF16, tag="sel")
        nc.vector.memset(ones_sb[:].bitcast(F32), 1.0)
        nc.vector.memset(ones_row[:], 1.0)
        nc.vector.memset(negc_sb[:], -EXP_C)
        nc.vector.memset(eps_sb[:], 1.0e-6)
        nc.vector.memset(ln8_sb[:], float(-0.5 * np.log(64.0)))
        nc.gpsimd.memset(sel_sb[:], 0.0)
        for j in range(4):
            nc.gpsimd.affine_select(
                out=sel_sb[:, j, :], in_=sel_sb[:, j, :],
                compare_op=ALU.not_equal, fill=1.0,
                base=-2 * j, pattern=[[-1, 2], [0, 64]], channel_multiplier=1)
        nonce_sb = pp.tile([1, _src_nonce()], F32, tag="nonce")
        ones_col = V[:, :, :, DH]
        nc.vector.memset(
            ones_col if DT == BF16 else ones_col.bitcast(F32), 1.0)

        # ---------------- Phase A: projections + RMSNorm ----------------
        with tc.tile_pool(name="xp", bufs=1) as px, \
                tc.tile_pool(name="wp", bufs=3) as pw, \
                tc.tile_pool(name="sqp", bufs=2) as psq, \
                tc.tile_pool(name="accp", bufs=1) as pacc, \
                tc.tile_pool(name="psA", bufs=4, space="PSUM") as psA, \
                tc.tile_pool(name="psS1", bufs=2, space="PSUM") as psS1, \
                tc.tile_pool(name="psBC", bufs=2, space="PSUM") as psBC:

            def _dma_w(dst, idx):
                # dc-pair chunks: full 128 partitions, 2KB contiguous lines
                for g in range(4):
                    nc.sync.dma_start(dst[:, 2 * g:2 * (g + 1), :],
                                      wT[:, idx, 2 * g:2 * (g + 1), :])

            x_sb = px.tile([P, NDC, TKV], DT, tag="x")
            wq0_sb = pw.tile([P, NDC, 512], DT, tag="wslice", name="wq0")
            wq1_sb = pw.tile([P, NDC, 512], DT, tag="wslice", name="wq1")
            wk0_sb = pw.tile([P, NDC, 512], DT, tag="wslice", name="wk0")
            # Startup critical path: the first Q-proj chain needs wq0[dc] and
            # x[:, dc, 512:] chunk by chunk. wq0 rides scalar's ring at per-dc
            # granularity (fast first arrival); x rides sync's ring as full
            # 2KB-line rows (efficient descriptors, x-lo arrives for free).
            # wq1/wk0 follow as 2KB-line dc-pairs split across both rings so
            # Q-wh1 (t~17us) and K (t~24us) never wait; everything less
            # urgent (wk1/wv/smalls) queues strictly behind them.
            # dc=0 pieces split across partition halves: half the descriptor
            # count per queue, so the very first matmul starts ~1us earlier
            for ph in range(2):
                pl = 64 * ph
                nc.scalar.dma_start(wq0_sb[pl:pl + 64, 0, :],
                                    wT[pl:pl + 64, 0, 0, :])
                nc.sync.dma_start(x_sb[pl:pl + 64, 0, 512:],
                                  xT[pl:pl + 64, 512:])
            for dc in range(1, NDC):
                nc.scalar.dma_start(wq0_sb[:, dc, :], wT[:, 0, dc, :])
                nc.sync.dma_start(x_sb[:, dc, 512:],
                                  xT[dc * P:(dc + 1) * P, 512:])
            # Everything below rides sync's ring: scalar's instruction queue
            # must stay clear for the PSUM eviction ACTs that gate psA reuse.
            for g in range(4):
                nc.sync.dma_start(wq1_sb[:, 2 * g:2 * (g + 1), :],
                                  wT[:, 1, 2 * g:2 * (g + 1), :])
            for g in range(4):
                nc.sync.dma_start(wk0_sb[:, 2 * g:2 * (g + 1), :],
                                  wT[:, 2, 2 * g:2 * (g + 1), :])
            # x low half (K group 2 / V back half, needed ~t+30us)
            for g in range(4):
                nc.sync.dma_start(x_sb[:, 2 * g:2 * (g + 1), :512],
                                  xT.rearrange("(dc p) t -> p dc t", p=P)
                                  [:, 2 * g:2 * (g + 1), :512])
            nc.sync.dma_start(gam_sb[:], gam[:])
            nc.sync.dma_start(nonce_sb[:], nonce[:])
            nc.sync.dma_start(qT[64:68], qext[:])
            nc.sync.dma_start(kT[64:68], kext[:])

            def _rms_reduce(proj, doff, acc):
                # rsqrt(mean+eps) = exp(-0.5*ln(ss/DIM + eps)); the 1/sqrt(dh)
                # score scale folds into the exp bias for q
                ss = psS1.tile([1, 512], F32, tag="ssq", name="ssq")
                nc.tensor.matmul(ss[:], ones_sb[:].bitcast(F32), acc[:],
                                 start=True, stop=True)
                a = psm.tile([1, 512], F32, tag="a")
                nc.scalar.activation(a[:], ss[:], AF.Ln,
                                     bias=eps_sb[:], scale=1.0 / DIM)
                y = psm.tile([1, 512], F32, tag="y")
                nc.scalar.activation(y[:], a[:], AF.Exp,
                                     bias=(ln8_sb[:] if proj == 0 else 0.0),
                                     scale=-0.5)
                return y

            def _rms_bcast(y, key):
                # broadcast over partitions via K=1 ones-matmul, then a bf16
                # SBUF copy so the normalize stt reads 16-bit SBUF operands
                bc = psBC.tile([P, 512], F32, tag="bc", name="bc")
                nc.tensor.matmul(bc[:], ones_row[:], y[:], start=True, stop=True)
                bcb = pacc.tile([P, 512], DT, tag=f"bcb{key}", name="bcb")
                nc.vector.tensor_copy(bcb[:], bc[:])
                return bcb

            def _norm_one(proj, h, doff, bcb, eng=None):
                dst = qT if proj == 0 else kT
                gap = gam_sb[:, proj * H + h: proj * H + h + 1]
                sl = dst[0:DH, h, doff:doff + 512]
                (eng or nc.vector).scalar_tensor_tensor(
                    sl, sl, gap, bcb[0:DH, :],
                    op0=ALU.mult, op1=ALU.mult,
                )

            # Q (tokens 512..1023 of the kv range) and K (all tokens);
            # K group (512,512) first so it only needs the priority x half.
            # dc-outer / ol-pair-inner: the first accumulation chain starts
            # as soon as the dc=0 chunks of w and x land (~256KB of DMA).
            sqacc = {}
            qnorm = []
            w_pre = {0: wq0_sb, 1: wq1_sb, 2: wk0_sb}
            for proj in range(2):
                dst = qT if proj == 0 else kT
                groups = [(TKV - TQ, 0)] if proj == 0 else [(512, 512), (0, 0)]
                for wh in range(2):
                    idx = proj * 2 + wh
                    if idx in w_pre:
                        w_sb = w_pre[idx]
                    else:
                        w_sb = pw.tile([P, NDC, 512], DT, tag="wslice")
                        _dma_w(w_sb, idx)
                    for (soff, doff) in groups:
                        for olp in range(2):
                            pss = [psA.tile([P, 512], F32, tag="projps",
                                            name="psp")
                                   for _ in range(2)]
                            for dc in range(NDC):
                                for oli in range(2):
                                    ol = olp * 2 + oli
                                    nc.tensor.matmul(
                                        pss[oli][:],
                                        w_sb[:, dc, ol * P:(ol + 1) * P],
                                        x_sb[:, dc, soff:soff + 512],
                                        start=(dc == 0), stop=(dc == NDC - 1),
                                    )
                            for oli in range(2):
                                odt = wh * 4 + olp * 2 + oli
                                ps = pss[oli]
                                # square first: the rms chain (ACT FIFO) must
                                # not queue behind the eviction copies
                                sq = psq.tile([P, 512], F32, tag="sq")
                                nc.scalar.activation(sq[:], ps[:], AF.Square)
                                nc.scalar.copy(
                                    dst[0:DH, 2 * odt, doff:doff + 512],
                                    ps[0:DH, :])
                                nc.vector.tensor_copy(
                                    dst[0:DH, 2 * odt + 1, doff:doff + 512],
                                    ps[DH:2 * DH, :])
                                key = (proj, doff)
                                if odt == 0:
                                    acc = pacc.tile([P, 512], F32,
                                                    tag=f"acc{proj}_{doff}",
                                                    name="acc")
                                    sqacc[key] = acc
                                    nc.gpsimd.tensor_copy(acc[:], sq[:])
                                else:
                                    nc.gpsimd.tensor_add(sqacc[key][:],
                                                         sqacc[key][:], sq[:])
                            # Q-norm stts interleave with K-proj so the DVE
                            # FIFO never backs up ahead of the PSUM evictions
                            for _ in range(2):
                                if proj == 1 and qnorm:
                                    h = qnorm.pop(0)
                                    _norm_one(0, h, 0, bcb_q)
                if proj == 0:
                    # Q's rms chain completes during K proj
                    y_q = _rms_reduce(0, 0, sqacc[(0, 0)])
                    bcb_q = _rms_bcast(y_q, "q")
                    qnorm = list(range(H))

            # V projection: [tok, head, dh]; vh innermost so consecutive
            # matmuls share the x-chunk stationary operand; K's rms chain AND
            # the 32 normalize stts interleave through the V matmul windows
            # so phase B can start right at the end of phase A
            wv_sb = []
            for vh in range(2):
                w_sb = pw.tile([P, NDC, 512], DT, tag="wslice")
                _dma_w(w_sb, 4 + vh)
                wv_sb.append(w_sb)
            y_k = {}
            bcb_k = {}
            knorm = [(h, doff) for h in range(H) for doff in (0, 512)]
            for vt, tt in enumerate((4, 5, 6, 7, 0, 1, 2, 3)):
                pss = [psA.tile([P, 512], F32, tag="projps", name="psv")
                       for _ in range(2)]
                for dc in range(NDC):
                    for vh in range(2):
                        nc.tensor.matmul(
                            pss[vh][:],
                            x_sb[:, dc, tt * P:(tt + 1) * P],
                            wv_sb[vh][:, dc, :],
                            start=(dc == 0), stop=(dc == NDC - 1),
                        )
                for vh in range(2):
                    nc.scalar.copy(
                        V[:, tt, vh * 8:(vh + 1) * 8, :DH],
                        pss[vh][:].rearrange("p (h c) -> p h c", c=DH),
                    )
                if vt == 1:
                    y_k[0] = _rms_reduce(1, 0, sqacc[(1, 0)])
                    y_k[512] = _rms_reduce(1, 512, sqacc[(1, 512)])
                    bcb_k[0] = _rms_bcast(y_k[0], "k0")
                    bcb_k[512] = _rms_bcast(y_k[512], "k512")
                if vt >= 2:
                    for ni in range(6):
                        if knorm:
                            h, doff = knorm.pop(0)
                            _norm_one(1, h, doff, bcb_k[doff])
            while knorm:
                h, doff = knorm.pop(0)
                _norm_one(1, h, doff, bcb_k[doff])

        if debug:
            nc.sync.dma_start(qT_d[:], qT[:])
            nc.sync.dma_start(kT_d[:], kT[:])
            nc.sync.dma_start(V_d[:], V[:])

        # wo tiles allocated now; the DMAs are issued mid-phase-B (hp==1) so
        # they never contend with the A->B boundary traffic
        pwo = tc.alloc_tile_pool(name="wop", bufs=2)
        wo_sbs = [pwo.tile([P, NDC, 512], DT, tag="wo", name=f"wo{oh}")
                  for oh in range(2)]

        def _wo_prefetch():
            for oh in range(2):
                for g in range(4):
                    nc.sync.dma_start(wo_sbs[oh][:, 2 * g:2 * (g + 1), :],
                                      woT[:, oh, 2 * g:2 * (g + 1), :])

        # ---------------- Phase B: attention (head pairs) ----------------
        with tc.tile_pool(name="maskp", bufs=1) as pm, \
                tc.tile_pool(name="ptp", bufs=4) as ppt, \
                tc.tile_pool(name="rnp", bufs=2) as prn, \
                tc.tile_pool(name="psO", bufs=1, space="PSUM") as psO, \
                tc.tile_pool(name="psS", bufs=1, space="PSUM") as psS:

            s16 = [pp.tile([8, TQ], F32, tag="s16a", name="s16a"),
                   pp.tile([8, TQ], F32, tag="s16b", name="s16b")]

            def _scores(hp):
                pts = {0: [], 1: []}
                for half in range(2):
                    ps_pair = [psS.tile([P, HW_HALF], F32, tag=f"sps{hi}", name=f"sps{hi}")
                               for hi in range(2)]
                    # hi-outer: hi=0's four key tiles complete first so its
                    # exp starts a full tile-time earlier on the ACT engine
                    for hi in range(2):
                        h = 2 * hp + hi
                        for ktl in range(4):
                            kt = half * 4 + ktl
                            off, wdt, qlo = KT_OFF[kt], KT_W[kt], KT_QLO[kt]
                            # 68-row contraction: head dims + ALiBi rank-4
                            nc.tensor.matmul(
                                ps_pair[hi][:, off:off + wdt],
                                kT[0:DH + 4, h, kt * P:(kt + 1) * P],
                                qT[0:DH + 4, h, qlo:qlo + wdt],
                                start=True, stop=True,
                            )
                    for hi in range(2):
                        pt = ppt.tile([P, HW_USED], DT, tag=f"pt{hi}")
                        nc.scalar.activation(pt[:], ps_pair[hi][:, :HW_USED],
                                             AF.Exp, bias=negc_sb[:])
                        _pt_fixups(nc, pt, half, hp)
                        if debug and hp == 0:
                            nc.sync.dma_start(pt_d[hi, :, half, :HW_USED],
                                              pt[:])
                        pts[hi].append(pt)
                return pts

            def _av(hp, pts):
                ps_o = [psO.tile([DH + 1, TQ], F32, tag=f"avps{hi}", name=f"avps{hi}")
                        for hi in range(2)]
                r = prn.tile([1, 2, TQ], F32, tag="r", name="r")
                for hi in range(2):
                    h = 2 * hp + hi
                    po = DH * hi
                    for i, (kt, q0, w) in enumerate(AV_PLAN):
                        c0 = KT_OFF[kt] + q0 - KT_QLO[kt]
                        nc.tensor.matmul(
                            ps_o[hi][:, q0:q0 + w],
                            V[:, kt, h, :],
                            pts[hi][kt // 4][:, c0:c0 + w],
                            start=(i == 0), stop=(i == len(AV_PLAN) - 1),
                            skip_group_check=True,
                        )
                    # stash the softmax denominator row
                    nc.vector.tensor_copy(r[:, hi, :], ps_o[hi][DH:DH + 1, :])
                    nc.vector.tensor_copy(attnT[po:po + DH, hp, :], ps_o[hi][:DH, :])
                # both heads' denominator rows -> partitions (2hp)%8..+1 of
                # s16 via one SBUF->SBUF DMA (engines cannot write partition
                # offsets other than 0/32/64)
                r0 = (2 * hp) % 8
                nc.sync.dma_start(s16[hp // 4][r0:r0 + 2, :], r[:])

            # software pipeline: scores(hp) issues before AV(hp-1), so the
            # PE covers the exp/fixup latency of hp while evicting hp-1 and
            # the ACT exp chain — phase B's bottleneck — never starves
            rc8b0 = pm.tile([8, TQ], BF16, tag="rcb0", name="rc8b0")
            pts_prev = None
            for hp in range(NDC):
                pts_cur = _scores(hp)
                if pts_prev is not None:
                    _av(hp - 1, pts_prev)
                pts_prev = pts_cur
                if hp == 1:
                    _wo_prefetch()
                if hp == 4:
                    # group 0 (8 heads) AV'd: batched reciprocal on 8
                    # partitions; the selection-matrix broadcast + normalize
                    # muls spread one pair per remaining hp. Group 1's chain
                    # is deferred into phase C.
                    rc8 = pm.tile([8, TQ], F32, tag="rc0", name="rc8")
                    nc.vector.reciprocal(rc8[:], s16[0][:])
                    nc.vector.tensor_copy(rc8b0[:], rc8[:])
                if hp >= 5:
                    hp2 = hp - 5
                    rb = psO.tile([P, TQ], F32,
                                  tag=f"avps{hp2 % 2}", name="rb")
                    nc.tensor.matmul(
                        rb[:], sel_sb[:, hp2, :], rc8b0[:],
                        start=True, stop=True)
                    nc.vector.tensor_mul(attnT[:, hp2, :],
                                         attnT[:, hp2, :], rb[:])
            _av(NDC - 1, pts_prev)
            for hp2 in range(3, 4):
                rb = psO.tile([P, TQ], F32,
                              tag=f"avps{hp2 % 2}", name="rb")
                nc.tensor.matmul(
                    rb[:], sel_sb[:, hp2, :], rc8b0[:],
                    start=True, stop=True)
                nc.vector.tensor_mul(attnT[:, hp2, :],
                                     attnT[:, hp2, :], rb[:])

        if debug:
            nc.sync.dma_start(aT_d[:], attnT[:])

        # ---------------- Phase C: output projection ----------------
        with tc.tile_pool(name="outp", bufs=1) as pout, \
                tc.tile_pool(name="psC", bufs=1, space="PSUM") as psC:
            out_sb = pout.tile([P, NQT, DIM], F32, tag="out")
            out_r = out.rearrange("(tt p) o -> p tt o", p=P)

            def _c_mms(pss, tt, adcs):
                # oh-major: the oh=0 chain finishes (copy + DMA kicks off)
                # while oh=1's matmuls still stream
                for oh in range(2):
                    for adc in adcs:
                        nc.tensor.matmul(
                            pss[oh][:],
                            attnT[:, adc, tt * P:(tt + 1) * P],
                            wo_sbs[oh][:, adc, :],
                            start=(adc == 0), stop=(adc == NDC - 1),
                            skip_group_check=True,
                        )
                    if adcs[-1] == NDC - 1:
                        _c_finish_oh(pss, tt, oh)

            def _c_finish_oh(pss, tt, oh):
                nc.vector.tensor_copy(
                    out_sb[:, tt, oh * 512:(oh + 1) * 512], pss[oh][:])
                # 4 DMAs per output half, alternating hwdge engines, so the
                # final drain rides 8 hw queues instead of 4
                for g in range(4):
                    eng = nc.sync if (g + oh) % 2 == 0 else nc.scalar
                    eng.dma_start(
                        out_r[32 * g:32 * (g + 1), tt,
                              oh * 512:(oh + 1) * 512],
                        out_sb[32 * g:32 * (g + 1), tt,
                               oh * 512:(oh + 1) * 512],
                        single_packet=True)

            # fronts for tt0-2 over the already-normalized first 8 heads
            # (6 PSUM banks) keep the PE streaming from the instant phase B
            # ends while group 1's reciprocal/broadcast chain completes
            psfront = {}
            for tt in range(3):
                psfront[tt] = [psC.tile([P, 512], F32, tag=f"cps{tt}_{oh}",
                                        name="psc") for oh in range(2)]
                _c_mms(psfront[tt], tt, range(4))

            rc8 = pout.tile([8, TQ], F32, tag="rc1", name="rc8")
            nc.vector.reciprocal(rc8[:], s16[1][:])
            rc8b = pout.tile([8, TQ], BF16, tag="rcb1", name="rc8b")
            nc.vector.tensor_copy(rc8b[:], rc8[:])
            for hp2 in range(4, 8):
                rb = psC.tile([P, TQ], F32, tag="rb", name="rb")
                nc.tensor.matmul(
                    rb[:], sel_sb[:, hp2 % 4, :], rc8b[:],
                    start=True, stop=True)
                nc.vector.tensor_mul(attnT[:, hp2, :],
                                     attnT[:, hp2, :], rb[:])

            for tt in range(3):
                _c_mms(psfront[tt], tt, range(4, NDC))
            # the final token tile runs in column-half chains so the first
            # half's copy+DMA streams while the second half computes — the
            # post-compute output flush is halved
            ps3 = [psC.tile([P, 512], F32, tag=f"cps0_{oh}",
                            name="psc3") for oh in range(2)]
            for oh in range(2):
                for ch in range(2):
                    c0 = 256 * ch
                    for adc in range(NDC):
                        nc.tensor.matmul(
                            ps3[oh][:, c0:c0 + 256],
                            attnT[:, adc, 3 * P:4 * P],
                            wo_sbs[oh][:, adc, c0:c0 + 256],
                            start=(adc == 0), stop=(adc == NDC - 1),
                            skip_group_check=True,
                        )
                    oc = oh * 512 + c0
                    nc.vector.tensor_copy(out_sb[:, 3, oc:oc + 256],
                                          ps3[oh][:, c0:c0 + 256])
                    # the very last half rides 4 queues so its post-compute
                    # transfer is as short as possible
                    ng = 4 if (oh, ch) == (1, 1) else 2
                    pw_ = P // ng
                    for g in range(ng):
                        eng = nc.sync if (g + ch + oh) % 2 == 0 else nc.scalar
                        eng.dma_start(
                            out_r[pw_ * g:pw_ * (g + 1), 3, oc:oc + 256],
                            out_sb[pw_ * g:pw_ * (g + 1), 3, oc:oc + 256])
        pwo.release()

    return nc


def _np_dt():
    return np.float32 if PREC == "fp32r" else ml_dtypes.bfloat16


def _build_ext(chunk0: bool):
    """qext [4,H,TQ]: (hi_q, lo_q, 1, 1);  kext [4,H,TKV]: (1, 1, k_hi, k_lo).
    hi/lo are a two-term bf16 split of -slope_h*(q_local+512); k_hi/k_lo of
    slope_h*k_local. Padded halo keys (chunk 0) get k_hi = -3e4 -> exp = 0."""
    np_dt = _np_dt()
    qe = np.zeros((4, H, TQ), np.float32)
    ke = np.zeros((4, H, TKV), np.float32)
    qpos = np.arange(TQ, dtype=np.float64) + 512.0
    kpos = np.arange(TKV, dtype=np.float64)
    qe[2:] = 1.0
    ke[:2] = 1.0
    qhi = np.zeros((H, TQ), np_dt)
    qlo = np.zeros((H, TQ), np_dt)
    khi = np.zeros((H, TKV), np_dt)
    klo = np.zeros((H, TKV), np_dt)
    for h in range(H):
        tq = (-SLOPES[h] * qpos).astype(np.float32)
        hi = tq.astype(np_dt)
        qhi[h] = hi
        qlo[h] = (tq - hi.astype(np.float32)).astype(np_dt)
        tk = (SLOPES[h] * kpos).astype(np.float32)
        if chunk0:
            tk[:512] = -3.0e4
        hi = tk.astype(np_dt)
        khi[h] = hi
        lo = (tk - hi.astype(np.float32))
        if chunk0:
            lo[:512] = 0.0
        klo[h] = lo.astype(np_dt)
    qe = qe.astype(np_dt)
    ke = ke.astype(np_dt)
    qe[0], qe[1] = qhi, qlo
    ke[2], ke[3] = khi, klo
    return np.ascontiguousarray(qe), np.ascontiguousarray(ke)


_NC = None
LAST = None  # BassKernelResults of the most recent run (exec_time_ns when traced)


def _get_nc():
    global _NC
    if _NC is None:
        _NC = _build_program()
    return _NC


def kernel(x, wq, wk, wv, wo, q_gamma, k_gamma):
    x = np.ascontiguousarray(np.asarray(x, np.float32))
    wq = np.asarray(wq, np.float32)
    wk = np.asarray(wk, np.float32)
    wv = np.asarray(wv, np.float32)
    wo = np.asarray(wo, np.float32)
    q_gamma = np.asarray(q_gamma, np.float32)
    k_gamma = np.asarray(k_gamma, np.float32)

    np_dt = _np_dt()
    # [in, out] -> [p, slice, dc, o]: per-partition-contiguous 8KB DMA lines
    wcat = np.concatenate([wq.T, wk.T, wv.T], axis=1)
    wT_host = np.ascontiguousarray(
        wcat.reshape(NDC, P, 6, 512).transpose(1, 2, 0, 3).astype(np_dt))
    woT_host = np.ascontiguousarray(
        wo.T.reshape(NDC, P, 2, 512).transpose(1, 2, 0, 3).astype(np_dt))
    gam_host = np.ascontiguousarray(np.concatenate(
        [q_gamma.reshape(H, DH).T, k_gamma.reshape(H, DH).T], axis=1))
    qe0, ke0 = _build_ext(True)
    qei, kei = _build_ext(False)

    in_maps = []
    for c in range(NCORES):
        b, j = divmod(c, 4)
        lo = j * TQ - WINDOW
        xs = x[b, max(0, lo): j * TQ + TQ, :]
        if lo < 0:
            xs = np.concatenate(
                [np.zeros((-lo, DIM), np.float32), xs], axis=0)
        in_maps.append({
            "nonce": np.zeros((1, _src_nonce()), np.float32),
            "xT": np.ascontiguousarray(xs.T.astype(np_dt)),
            "wT": wT_host,
            "woT": woT_host,
            "gam": gam_host,
            "qext": qe0 if j == 0 else qei,
            "kext": ke0 if j == 0 else kei,
        })

    global LAST
    trace = bool(int(os.environ.get("KERNEL_TRACE", "0") or 0))
    try:
        LAST = run_bass_kernel_spmd(
            _get_nc(), in_maps, list(range(NCORES)), trace=trace)
    except Exception:
        # a previously-wedged device surfaces as NRT_EXEC_UNIT_UNRECOVERABLE
        # on the first touch; reset the accelerator once and retry
        try:
            lib = ctypes.CDLL("/opt/axon/libaxon_pjrt.so")
            lib.axon_reset.restype = ctypes.c_int64
            import jax
            jax.devices()
            lib.axon_reset()
        except Exception:
            pass
        LAST = run_bass_kernel_spmd(
            _get_nc(), in_maps, list(range(NCORES)), trace=trace)

    full = np.empty((B, T, DIM), np.float32)
    for c in range(NCORES):
        b, j = divmod(c, 4)
        full[b, j * TQ:(j + 1) * TQ, :] = LAST.results[c]["out"]
    return full

